# revision 1
# baseline (speedup 1.0000x reference)
"""Trainium2 Bass kernel for nn_CrossAttention1d (B=8, C=768, N=256, H=12, D=64).

Math (per batch b), algebraically equal to the reference but avoiding the
[3072, 3072] attention matrix via associativity:

    cp_full = W_proj @ cross_b + b_proj              [C, N]
    CP = cp_full.reshape(D, H*N)   (pure reshape)
    Xc = cross_b.reshape(D, H*N)   (pure reshape)
    K  = CP @ Xc^T                                   [D, D]
    X  = x_ori_b.reshape(D, H*N)
    OT = scale * K^T @ X                             [D, H*N]   (= O^T)
    out2T[h*64+d, n] = OT[d, n*12+h]                 [C, N]
    yT = W_dep @ out2T + b_dep                       [C, N]
    out_b = x_ori_b + yT

Sharding: data-parallel over batch, one batch per NeuronCore (8 cores).

On-chip schedule (per core):
  - proj computed transposed: cpT[n, o] = sum_c cross[c, n] wpT[c, o] (+ bias
    via a rank-1 K=1 matmul) so the K-matmul lhsT is a stride-12 free-dim
    slice of cpT (no transpose needed for CP).
  - crossT via 12 PE transposes (K-matmul rhs, also stride-12 slices).
  - K accumulated over 24 [128]x[64]x[64] matmuls; the attention scale is
    folded in during the PSUM->SBUF eviction, which also duplicates K to
    partitions [64:128] so OT matmuls can run on either partition half.
  - x loaded as [128, 1536] (p = half*64 + d, f = m - half*1536) for full
    DMA port width.
  - OT produced parity-split: OT2[d, t] = OT[d, 2t], OT2[64+d, t] = OT[d, 2t+1]
    by running each OT matmul twice with even/odd strided rhs, writing PSUM
    partitions [0:64] and [64:128].  The deproj rhs for c'-chunk j is then the
    single AP OT2[:, j::6] at full K=128.
  - deproj + b_dep rank-1 + residual add, store.

dtype variants: fp32 everywhere, or bf16 for the big DMA streams (weights,
cross, and the on-chip OT2) with fp32 PSUM accumulation throughout.
"""

import numpy as np

import concourse.bacc as bacc
import concourse.mybir as mybir
import concourse.tile as tile
from concourse.bass_utils import run_bass_kernel_spmd
from concourse.masks import make_identity

B, C, N = 8, 768, 256
H, D = 12, 64
M = H * N  # 3072
SCALE = float(D) ** -0.5
N_CORES = 8
F32 = mybir.dt.float32
BF16 = mybir.dt.bfloat16

USE_BF16 = True

_built_nc = None


def emit(tc, nc, xq, xr, cr, wp, wd, bp, bd, out, bf16):
    """Emit one batch's worth of IR. DRAM handle args."""
    add = mybir.AluOpType.add
    Copy = mybir.ActivationFunctionType.Copy
    WDT = BF16 if bf16 else F32  # weight / cross / ot2 storage dtype

    with tc.tile_pool(name="sb", bufs=1) as sb:
        # ---- constants -------------------------------------------------
        ident = sb.tile([128, 128], WDT)
        make_identity(nc, ident[:])
        ones = sb.tile([1, 256], WDT)
        nc.gpsimd.memset(ones[:], 1.0)

        # ---- input DMAs (all fully contiguous, host-permuted) ----------
        cross_sb = sb.tile([128, 6 * N], WDT)
        nc.sync.dma_start(cross_sb[:], cr.ap())

        wp_sb = sb.tile([128, 6 * C], WDT)
        nc.sync.dma_start(wp_sb[:], wp.ap())

        x_sb = sb.tile([128, M // 2], WDT)
        nc.sync.dma_start(x_sb[:], xq.ap())

        bp_sb = sb.tile([1, C], WDT)
        nc.sync.dma_start(bp_sb[:], bp.ap())
        bd_sb = sb.tile([1, C], WDT)
        nc.sync.dma_start(bd_sb[:], bd.ap())

        wd_sb = sb.tile([128, 6 * C], WDT)
        nc.sync.dma_start(wd_sb[:], wd.ap())

        xr_sb = sb.tile([128, 6 * N], WDT)
        nc.sync.dma_start(xr_sb[:], xr.ap())

        # ---- working SBUF tiles ---------------------------------------
        cpT_sb = sb.tile([128, 2 * C], F32)   # [n-chunk p, ni*768 + o]
        crT_sb = sb.tile([128, 2 * C], F32)   # [n-chunk p, ni*768 + c]
        k_sb = sb.tile([128, 64], WDT)        # scale * K, duplicated halves
        ot2 = sb.tile([128, M // 2], WDT)     # parity-split OT
        out_sb = sb.tile([128, 6 * N], WDT)

        # ---- proj (transposed) + crossT --------------------------------
        with (
            tc.tile_pool(name="ppj", bufs=4, space="PSUM") as ppj,
            tc.tile_pool(name="ptr", bufs=2, space="PSUM") as ptr,
        ):
            for ni in range(2):
                for oj in range(2):
                    ps = ppj.tile([128, 384], F32)
                    for t in range(6):
                        nc.tensor.matmul(
                            ps[:],
                            cross_sb[:, t * N + ni * 128: t * N + ni * 128 + 128],
                            wp_sb[:, t * C + oj * 384: t * C + oj * 384 + 384],
                            start=(t == 0),
                            stop=False,
                        )
                    # bias: cpT[n, o] += 1 * b_proj[o]
                    nc.tensor.matmul(
                        ps[:],
                        ones[0:1, 0:128],
                        bp_sb[0:1, oj * 384:(oj + 1) * 384],
                        start=False,
                        stop=True,
                    )
                    nc.vector.tensor_copy(
                        cpT_sb[:, ni * C + oj * 384: ni * C + oj * 384 + 384], ps[:]
                    )

            # crossT: 12 PE transposes of [128, 128] blocks
            for t in range(6):
                for ni in range(2):
                    pt = ptr.tile([128, 128], WDT)
                    nc.tensor.transpose(
                        pt[:],
                        cross_sb[:, t * N + ni * 128: t * N + ni * 128 + 128],
                        ident[:],
                    )
                    nc.scalar.activation(
                        crT_sb[:, ni * C + t * 128: ni * C + t * 128 + 128],
                        pt[:],
                        Copy,
                    )

        # ---- K / OT / deproj -------------------------------------------
        with (
            tc.tile_pool(name="pk", bufs=1, space="PSUM") as pk,
            tc.tile_pool(name="pot", bufs=3, space="PSUM") as pot,
            tc.tile_pool(name="py", bufs=2, space="PSUM") as py,
        ):
            # K[d', d] accumulated over (h, ni)
            kps = pk.tile([64, 64], F32)
            cpT_v = cpT_sb[:].rearrange("p (c d h) -> p c h d", c=2, h=H)
            crT_v = crT_sb[:].rearrange("p (c d h) -> p c h d", c=2, h=H)
            first = True
            for h in range(H):
                for ni in range(2):
                    nc.tensor.matmul(
                        kps[:],
                        cpT_v[:, ni, h],
                        crT_v[:, ni, h],
                        start=first,
                        stop=(h == H - 1 and ni == 1),
                    )
                    first = False
            # fold the attention scale in; duplicate K onto both halves
            nc.scalar.activation(k_sb[0:64, :], kps[:], Copy, scale=SCALE)
            nc.scalar.activation(k_sb[64:128, :], kps[:], Copy, scale=SCALE)

            # OT parity-split: even m -> partitions [0:64], odd m -> [64:128]
            x_v = x_sb[:].rearrange("p (t par) -> p par t", par=2)  # f = 2t+par
            for j in range(6):
                half, sub = j // 3, j % 3
                hb = half * 64
                po = pot.tile([128, 256], F32)
                nc.tensor.matmul(
                    po[0:64, :],
                    k_sb[hb:hb + 64, :],
                    x_v[hb:hb + 64, 0, sub * 256:(sub + 1) * 256],
                    start=True, stop=True,
                )
                nc.tensor.matmul(
                    po[64:128, :],
                    k_sb[hb:hb + 64, :],
                    x_v[hb:hb + 64, 1, sub * 256:(sub + 1) * 256],
                    start=True, stop=True,
                )
                nc.vector.tensor_copy(ot2[:, j * 256:(j + 1) * 256], po[:])

            # deproj + b_dep + residual
            ot2_v = ot2[:].rearrange("p (t six) -> p six t", six=6)
            for oi in range(6):
                yps = py.tile([128, 256], F32)
                for j in range(6):
                    nc.tensor.matmul(
                        yps[:],
                        wd_sb[:, j * C + oi * 128: j * C + oi * 128 + 128],
                        ot2_v[:, j],
                        start=(j == 0),
                        stop=False,
                    )
                nc.tensor.matmul(
                    yps[:],
                    bd_sb[0:1, oi * 128:(oi + 1) * 128],
                    ones[0:1, 0:256],
                    start=False,
                    stop=True,
                )
                nc.vector.tensor_tensor(
                    out_sb[:, oi * N:(oi + 1) * N],
                    yps[:],
                    xr_sb[:, oi * N:(oi + 1) * N],
                    add,
                )

        # ---- store -----------------------------------------------------
        for s in range(3):
            nc.sync.dma_start(
                out.ap()[:, s * 512:(s + 1) * 512], out_sb[:, s * 512:(s + 1) * 512]
            )


def _declare(nc, bf16):
    WDT = BF16 if bf16 else F32
    # all inputs host-pre-permuted into the exact SBUF layout -> every DMA is
    # one fully contiguous block at HBM line rate
    xq = nc.dram_tensor("xq", [128, M // 2], WDT, kind="ExternalInput")
    xr = nc.dram_tensor("xr", [128, 6 * N], WDT, kind="ExternalInput")
    cr = nc.dram_tensor("cr", [128, 6 * N], WDT, kind="ExternalInput")
    wp = nc.dram_tensor("wp", [128, 6 * C], WDT, kind="ExternalInput")
    wd = nc.dram_tensor("wd", [128, 6 * C], WDT, kind="ExternalInput")
    bp = nc.dram_tensor("bp", [1, C], WDT, kind="ExternalInput")
    bd = nc.dram_tensor("bd", [1, C], WDT, kind="ExternalInput")
    out = nc.dram_tensor("out", [128, 6 * N], WDT, kind="ExternalOutput")
    return xq, xr, cr, wp, wd, bp, bd, out


def build(bf16=USE_BF16):
    nc = bacc.Bacc("TRN2", target_bir_lowering=False, debug=False)
    args = _declare(nc, bf16)
    with tile.TileContext(nc) as tc:
        emit(tc, nc, *args, bf16)
    nc.compile()
    return nc


def build_loop(reps, bf16=USE_BF16):
    """Kernel body wrapped in a hardware For loop, for wall-clock timing."""
    nc = bacc.Bacc("TRN2", target_bir_lowering=False, debug=False)
    args = _declare(nc, bf16)
    with tile.TileContext(nc) as tc:
        with tc.For_i(0, reps, 1, hint_engines=(mybir.EngineType.PE,)):
            emit(tc, nc, *args, bf16)
    nc.compile()
    return nc


def make_in_maps(x_ori, cross, W_proj, b_proj, W_dep, b_dep, bf16=USE_BF16):
    import ml_dtypes

    wdt = ml_dtypes.bfloat16 if bf16 else np.float32
    x_ori = np.asarray(x_ori, np.float32)
    cross = np.asarray(cross, np.float32)

    def w_perm(w):  # [C, C] W^T -> [128, 4608] SBUF layout
        return np.ascontiguousarray(
            w.T.reshape(2, 3, 128, C).transpose(2, 0, 1, 3).reshape(128, 6 * C)
            .astype(wdt)
        )

    def tn_perm(a):  # [C, N] -> [128, (t n)]
        return np.ascontiguousarray(
            a.reshape(6, 128, N).transpose(1, 0, 2).reshape(128, 6 * N).astype(wdt)
        )

    def xq_perm(a):  # [C, N] -> [128, 1536], p = half*64+d, f = m - half*1536
        return np.ascontiguousarray(
            a.reshape(D, 2, M // 2).transpose(1, 0, 2).reshape(128, M // 2)
            .astype(wdt)
        )

    wp = w_perm(np.asarray(W_proj, np.float32))
    wd = w_perm(np.asarray(W_dep, np.float32))
    bp = np.ascontiguousarray(np.asarray(b_proj, np.float32).reshape(1, C), wdt)
    bd = np.ascontiguousarray(np.asarray(b_dep, np.float32).reshape(1, C), wdt)
    return [
        {
            "xq": xq_perm(x_ori[b]),
            "xr": tn_perm(x_ori[b]),
            "cr": tn_perm(cross[b]),
            "wp": wp,
            "wd": wd,
            "bp": bp,
            "bd": bd,
        }
        for b in range(B)
    ]


def unpermute_out(o):  # [128, (t n)] -> [C, N]
    return np.asarray(o, np.float32).reshape(128, 6, N).transpose(1, 0, 2).reshape(C, N)


def kernel(**inputs):
    global _built_nc
    if _built_nc is None:
        _built_nc = build()
    nc = _built_nc
    in_maps = make_in_maps(
        inputs["x_ori"], inputs["cross"], inputs["W_proj"],
        inputs["b_proj"], inputs["W_dep"], inputs["b_dep"],
    )
    res = run_bass_kernel_spmd(nc, in_maps, list(range(N_CORES)))
    out = np.stack([unpermute_out(res.results[c]["out"]) for c in range(N_CORES)])
    return out.astype(np.float32)



# revision 13
# speedup vs baseline: 2.1314x; 2.1314x over previous
"""Trainium2 Bass kernel for nn_CrossAttention1d (B=8, C=768, N=256, H=12, D=64).

Math (per batch b), algebraically equal to the reference but avoiding the
[3072, 3072] attention matrix via associativity:

    cp  = W_proj @ cross_b + b_proj                  [C, N]
    CP  = cp.reshape(D, H*N)      (pure reshape)
    Xc  = cross_b.reshape(D, H*N) (pure reshape)
    K   = CP @ Xc^T                                  [D, D]
    X   = x_ori_b.reshape(D, H*N)
    OT  = scale * K^T @ X                            [D, H*N]   (= O^T)
    out2T[h*64+d, n] = OT[d, n*12+h]                 [C, N]
    yT  = W_dep @ out2T + b_dep                      [C, N]
    out_b = x_ori_b + yT

Sharding: data-parallel over batch, one batch per NeuronCore (8 cores).

Implementation notes (per core / per iteration):
  - bf16 everywhere on the compute path (fp8 was tried: its ~3.6% per-tensor
    quantization error propagates ~1:1 through each linear stage and lands at
    ~7e-2 rel error, over the 2e-2 budget).  The residual copy of x is fp8:
    its error is damped 16x by the output magnitude.
  - cross^T (K-matmul rhs) is a host-side permutation of cross and is DMA'd,
    killing the baseline's 12 PE transposes + 12 evictions.
  - Inputs arrive in 4 DMAs (vs baseline 7): one combined [128, 12288] bf16
    tensor (cross | cross^T | W_proj^T | W_dep^T), x as [64, 3072] bf16,
    the fp8 residual [128, 1536], and biases [1, 1536].  One bf16 store.
    Input DMAs issue on the SP queue, the store on the Pool queue, so a
    store waiting on compute never head-blocks the next set's loads.
  - NSETS independent buffer sets via persistent tile pools with
    bufs=NSETS rotation, so DMA-in of later sets overlaps compute of the
    current one.  (Per-emit pools would be re-allocated LIFO at the same
    addresses, and the resulting WAR semaphores serialize iterations.)
  - OT is computed via stride-6 column classes: po[0:64] <- columns
    {2r, 2r+6}, po[64:128] <- {2r+1, 2r+7}, each a single [64, 512] matmul;
    the eviction de-interleaves (u = 2n+s) into out2T layout directly.
  - PSUM evictions: GPSIMD/Pool cannot read PSUM on real HW, so they are
    split between Activation (cpT, K) and DVE (OT, deproj+residual); both
    stay well under the DMA bound.  Pool handles the store DMA.

Steady state is DMA-bound: ~4.1 MB / 360 GB/s ~ 11.5 us/iter, with the PE
close behind (~10.9 us: two 768x768x256 bf16 GEMMs + small stages).
"""

import numpy as np

import concourse.bacc as bacc
import concourse.mybir as mybir
import concourse.tile as tile
from concourse.bass_utils import run_bass_kernel_spmd

B, C, N = 8, 768, 256
H, D = 12, 64
M = H * N  # 3072
SCALE = float(D) ** -0.5
N_CORES = 8
F32 = mybir.dt.float32
BF16 = mybir.dt.bfloat16
FP8 = mybir.dt.float8e4

NSETS = 4  # pipeline depth (independent buffer sets per loop body)

# big-tensor column offsets
O_CROSS = 0
O_CRT = 1536
O_WP = 3072
O_WD = 7680
BIGW = 12288

_built_nc = None


def emit(tc, nc, dram, ones, sbd, psum):
    """Emit one batch-iteration using rotating tiles from persistent pools."""
    Copy = mybir.ActivationFunctionType.Copy
    add = mybir.AluOpType.add
    big, xp, xr, bias, out = dram
    ppj, pk, pot, py = psum

    # ---- input DMAs (host-permuted, fully contiguous; SP queue) --------
    big_sb = sbd.tile([128, BIGW], BF16, name="big_sb")
    nc.sync.dma_start(big_sb[:], big.ap())
    xp_sb = sbd.tile([64, M], BF16, name="xp_sb")
    nc.sync.dma_start(xp_sb[:], xp.ap())
    xr_sb = sbd.tile([128, 1536], BF16, name="xr_sb")
    nc.sync.dma_start(xr_sb[:], xr.ap())
    bias_sb = sbd.tile([1, 1536], BF16, name="bias_sb")
    nc.sync.dma_start(bias_sb[:], bias.ap())

    # ---- working tiles -------------------------------------------------
    cpT = sbd.tile([128, 1536], BF16, name="cpT")    # [n%128, ni*768+(d'*12+h)]
    k_sb = sbd.tile([64, 64], BF16, name="k_sb")     # SCALE * K
    ot2 = sbd.tile([128, 1536], BF16, name="ot2")    # [(h&1)*64+d, (h>>1)*256+n]
    out_sb = sbd.tile([128, 1536], BF16, name="out_sb")

    # ---- proj: cpT[n, o] = (cross^T Wp^T + bp)[n, o] -------------------
    for ni in range(2):
        for oj in range(2):
            ps = ppj.tile([128, 384], F32, name="ps")
            for t in range(6):
                nc.tensor.matmul(
                    ps[:],
                    big_sb[:, O_CROSS + t * N + ni * 128:
                           O_CROSS + t * N + ni * 128 + 128],
                    big_sb[:, O_WP + t * C + oj * 384:
                           O_WP + t * C + oj * 384 + 384],
                    start=(t == 0), stop=False,
                )
            nc.tensor.matmul(
                ps[:], ones[0:1, 0:128],
                bias_sb[0:1, oj * 384:(oj + 1) * 384],
                start=False, stop=True,
            )
            nc.scalar.activation(
                cpT[:, ni * C + oj * 384: ni * C + oj * 384 + 384],
                ps[:], Copy)

    # ---- K[d', d] = sum_{ni,h,p} cpT[p, ni, d'*12+h] crT[p, ni, d*12+h]
    kps = pk.tile([64, 64], F32, name="kps")
    first = True
    for ni in range(2):
        for h in range(H):
            nc.tensor.matmul(
                kps[:],
                cpT[:, ni * C + h: ni * C + h + 63 * 12 + 1: 12],
                big_sb[:, O_CRT + ni * C + h:
                       O_CRT + ni * C + h + 63 * 12 + 1: 12],
                start=first, stop=(ni == 1 and h == H - 1),
            )
            first = False
    nc.scalar.activation(k_sb[:], kps[:], Copy, scale=SCALE)

    # ---- OT: po[p6*64+d, u] = OT[d, m], m = 2r+p6+6u -------------------
    # stride-6 column class {h0, h0+6} interleaves as u = 2n+s
    for r in range(3):
        po = pot.tile([128, 512], F32, name="po")
        nc.tensor.matmul(po[0:64, :], k_sb[:], xp_sb[:, 2 * r::6],
                         start=True, stop=True)
        nc.tensor.matmul(po[64:128, :], k_sb[:], xp_sb[:, 2 * r + 1::6],
                         start=True, stop=True)
        src = po[:].rearrange("p (n s) -> p s n", s=2)
        dst = ot2[:, r * 256: r * 256 + 1024].rearrange(
            "p (s n) -> p s n", s=4)[:, 0::3]
        nc.vector.tensor_copy(dst, src)

    # ---- deproj + bias + residual --------------------------------------
    for oi in range(6):
        yps = py.tile([128, 256], F32, name="yps")
        for q in range(6):
            nc.tensor.matmul(
                yps[:],
                big_sb[:, O_WD + q * C + oi * 128: O_WD + q * C + oi * 128 + 128],
                ot2[:, q * 256:(q + 1) * 256],
                start=(q == 0), stop=False,
            )
        nc.tensor.matmul(
            yps[:], bias_sb[0:1, 768 + oi * 128: 768 + oi * 128 + 128],
            ones[0:1, 0:256],
            start=False, stop=True,
        )
        nc.vector.tensor_tensor(
            out_sb[:, oi * 256:(oi + 1) * 256],
            yps[:], xr_sb[:, oi * 256:(oi + 1) * 256], add)

    # ---- store (Pool queue: don't head-block the SP input stream) ------
    nc.gpsimd.dma_start(out.ap(), out_sb[:])


def _declare(nc, n_sets):
    """Inputs are shared across pipeline slots (read-only); out is per-slot."""
    big = nc.dram_tensor("big", [128, BIGW], BF16, kind="ExternalInput")
    xp = nc.dram_tensor("xp", [64, M], BF16, kind="ExternalInput")
    xr = nc.dram_tensor("xr", [128, 1536], BF16, kind="ExternalInput")
    bias = nc.dram_tensor("bias", [1, 1536], BF16, kind="ExternalInput")
    args = []
    for s in range(n_sets):
        sfx = f"_{s}" if n_sets > 1 else ""
        out = nc.dram_tensor(f"out{sfx}", [128, 1536], BF16, kind="ExternalOutput")
        args.append((big, xp, xr, bias, out))
    return args


def _pools(tc, nc, n_sets):
    const = tc.alloc_tile_pool(name="const", bufs=1)
    ones = const.tile([1, 256], BF16)
    nc.gpsimd.memset(ones[:], 1.0)
    sbd = tc.alloc_tile_pool(name="sbd", bufs=n_sets)
    ppj = tc.alloc_tile_pool(name="ppj", bufs=2, space="PSUM")
    pk = tc.alloc_tile_pool(name="pk", bufs=1, space="PSUM")
    pot = tc.alloc_tile_pool(name="pot", bufs=2, space="PSUM")
    py = tc.alloc_tile_pool(name="py", bufs=2, space="PSUM")
    pools = (const, sbd, ppj, pk, pot, py)
    return pools, ones, sbd, (ppj, pk, pot, py)


def build():
    nc = bacc.Bacc("TRN2", target_bir_lowering=False, debug=False)
    args = _declare(nc, 1)
    with tile.TileContext(nc) as tc:
        pools, ones, sbd, psum = _pools(tc, nc, 1)
        emit(tc, nc, args[0], ones, sbd, psum)
        for p in reversed(pools):
            p.release()
    nc.compile()
    return nc


def build_flat(n_iters):
    """n_iters unrolled (no hardware loop) - for timeline sim."""
    nc = bacc.Bacc("TRN2", target_bir_lowering=False, debug=False)
    args = _declare(nc, NSETS)
    with tile.TileContext(nc) as tc:
        pools, ones, sbd, psum = _pools(tc, nc, NSETS)
        for it in range(n_iters):
            emit(tc, nc, args[it % NSETS], ones, sbd, psum)
        for p in reversed(pools):
            p.release()
    nc.compile()
    return nc


def build_loop(reps):
    """Kernel body wrapped in a hardware For loop, for wall-clock timing."""
    assert reps % NSETS == 0, f"reps must be divisible by {NSETS}"
    nc = bacc.Bacc("TRN2", target_bir_lowering=False, debug=False)
    args = _declare(nc, NSETS)
    with tile.TileContext(nc) as tc:
        pools, ones, sbd, psum = _pools(tc, nc, NSETS)
        with tc.For_i(0, reps // NSETS, 1, hint_engines=(mybir.EngineType.PE,)):
            for s in range(NSETS):
                emit(tc, nc, args[s], ones, sbd, psum)
        for p in reversed(pools):
            p.release()
    nc.compile()
    return nc


def make_in_maps(x_ori, cross, W_proj, b_proj, W_dep, b_dep):
    import ml_dtypes

    fp8 = ml_dtypes.float8_e4m3
    bf16 = ml_dtypes.bfloat16
    x_ori = np.asarray(x_ori, np.float32)
    cross = np.asarray(cross, np.float32)

    def w_perm(w):  # [o, c] -> [128, (t o)] of W^T
        return w.T.reshape(6, 128, C).transpose(1, 0, 2).reshape(128, 4608)

    wpP = w_perm(np.asarray(W_proj, np.float32))
    wdP = w_perm(np.asarray(W_dep, np.float32))
    biasP = np.concatenate(
        [np.asarray(b_proj, np.float32),
         np.asarray(b_dep, np.float32)]).reshape(1, 1536).astype(bf16)

    maps = []
    for b in range(B):
        cr, xo = cross[b], x_ori[b]
        crossP = cr.reshape(6, 128, N).transpose(1, 0, 2).reshape(128, 1536)
        crT = cr.T.reshape(2, 128, C).transpose(1, 0, 2).reshape(128, 1536)
        big = np.ascontiguousarray(
            np.concatenate([crossP, crT, wpP, wdP], axis=1)).astype(bf16)
        xpP = np.ascontiguousarray(xo.reshape(D, M)).astype(bf16)
        xrP = np.ascontiguousarray(
            xo.reshape(6, 128, N).transpose(1, 0, 2).reshape(128, 1536)
        ).astype(bf16)
        maps.append({"big": big, "xp": xpP, "xr": xrP, "bias": biasP})
    return maps


def unpermute_out(o):  # [128, (u n)] -> [C, N]
    return np.asarray(o, np.float32).reshape(128, 6, N).transpose(1, 0, 2).reshape(C, N)


def kernel(**inputs):
    global _built_nc
    if _built_nc is None:
        _built_nc = build()
    nc = _built_nc
    in_maps = make_in_maps(
        inputs["x_ori"], inputs["cross"], inputs["W_proj"],
        inputs["b_proj"], inputs["W_dep"], inputs["b_dep"],
    )
    res = run_bass_kernel_spmd(nc, in_maps, list(range(N_CORES)))
    out = np.stack([unpermute_out(res.results[c]["out"]) for c in range(N_CORES)])
    return out.astype(np.float32)
